# revision 1
# baseline (speedup 1.0000x reference)
"""Trainium2 Bass kernel for nn_AttentionSelector (segment softmax attention).

Math shortcut used throughout: since
    logits = segment_sum(w * repre) @ relation_mat.T + bias
and matmul is linear, we can first compute P = repre @ relation_mat.T ([N,53])
and do the entire segment softmax + weighted reduction in 53-dim space:
    x_i      = P[i, labels[i]]
    w_i      = segment_softmax(x)_i
    logits_b = sum_{i in bag b} w_i * P[i, :] + bias

Device pipeline (per core, bags sharded 3125/core):
  Stage A: stream repre^T (the 552MB roofline), 6 accumulating fp32r matmuls
           per 512-row block -> P^T in PSUM; PE-transpose to row-major P;
           extract x via iota/is_equal mask + fused multiply-reduce.
  Stage B: ragged segment softmax with no gathers: forward+backward
           *segmented scans* (tensor_tensor_scan with reset masks, reversed
           APs for the backward direction) + a one-step cross-partition carry
           fixup through tiny PE transposes.
  Stage C: weighted segment-sum via per-128-row-chunk one-hot matmul
           H.T @ P with H[i,j] = (seg_local_i == j) * w_i built by a single
           fused tensor_scalar; host compacts the <=2 partial slots per bag.
"""
import math
import os
import sys

for _p in ("/opt/trn_rl_repo", "/opt/trn_rl_repo/concourse", "/opt/pypackages"):
    if _p not in sys.path:
        sys.path.insert(0, _p)

import numpy as np

N_TOTAL = 200000
NUM_BAGS = 25000
DIM = 690
REL = 53
NCORES = 8
KCH = 115          # contraction chunk (DIM = NK * KCH)
NK = 6
BSHIFT = 256.0     # positivity offset for the segmented max scan
MM_DTYPE = os.environ.get("KERNEL_MM_DTYPE", "float32r")

LAST_RESULTS = None
_PROGRAM_CACHE = {}


def _build_program(Rpad, dt_mm=MM_DTYPE, debug_out=False):
    from concourse import bacc, mybir
    import concourse.tile as tile
    from concourse.masks import make_identity

    f32 = mybir.dt.float32
    dtmm = getattr(mybir.dt, dt_mm)
    Alu = mybir.AluOpType
    NJ = Rpad // 512
    NCH = Rpad // 128
    C = NCH

    nc = bacc.Bacc("TRN2", target_bir_lowering=False, debug=False,
                   enable_asserts=False)

    with tile.TileContext(nc) as tc:
        with tc.tile_pool(name="dram", bufs=1, space="DRAM") as dram, \
             tc.tile_pool(name="consts", bufs=1) as consts, \
             tc.tile_pool(name="xt", bufs=3) as xtp, \
             tc.tile_pool(name="ptsb", bufs=2) as ptsbp, \
             tc.tile_pool(name="small", bufs=4) as smallp, \
             tc.tile_pool(name="big", bufs=1) as bigp, \
             tc.tile_pool(name="hp", bufs=3) as hbp, \
             tc.tile_pool(name="segb", bufs=1) as segp, \
             tc.tile_pool(name="pt_ps", bufs=2, space="PSUM") as ptps, \
             tc.tile_pool(name="tr_ps", bufs=3, space="PSUM") as trps:

            xT_d = dram.tile([DIM, Rpad], dtmm, kind="ExternalInput", name="xT", uniquify=False)
            wm_d = dram.tile([KCH, NK, REL], dtmm, kind="ExternalInput", name="wm", uniquify=False)
            lab_d = dram.tile([Rpad], f32, kind="ExternalInput", name="labf", uniquify=False)
            seg_d = dram.tile([Rpad], f32, kind="ExternalInput", name="segloc", uniquify=False)
            cf_d = dram.tile([128, C], f32, kind="ExternalInput", name="cf", uniquify=False)
            cb_d = dram.tile([128, C], f32, kind="ExternalInput", name="cb", uniquify=False)
            att_d = dram.tile([NCH, 128, REL], f32, kind="ExternalOutput",
                              name="attstage", uniquify=False)
            _dbg = dict(kind="ExternalOutput", uniquify=False) if debug_out else {}
            xlin_d = dram.tile([128, C], f32, name="xlin", **_dbg)
            wlin_d = dram.tile([128, C], f32, name="wlin", **_dbg)

            # constants
            ident = consts.tile([128, 128], f32, name="ident", tag="ident")
            make_identity(nc, ident[:])
            io53_i = consts.tile([128, REL], mybir.dt.int32, name="io53i", tag="io53i")
            nc.gpsimd.iota(io53_i[:], pattern=[[1, REL]], base=0, channel_multiplier=0)
            io53 = consts.tile([128, REL], f32, name="io53", tag="io53")
            nc.vector.tensor_copy(io53[:], io53_i[:])
            io128_i = consts.tile([128, 128], mybir.dt.int32, name="io128i", tag="io128i")
            nc.gpsimd.iota(io128_i[:], pattern=[[1, 128]], base=0, channel_multiplier=0)
            io128 = consts.tile([128, 128], f32, name="io128", tag="io128")
            nc.vector.tensor_copy(io128[:], io128_i[:])

            wm_sb = consts.tile([KCH, NK, REL], dtmm, name="wm_sb", tag="wm_sb")
            nc.sync.dma_start(wm_sb[:], wm_d[:])
            laball = consts.tile([128, NCH], f32, name="laball", tag="laball")
            nc.sync.dma_start(laball[:], lab_d[:].rearrange("(c p) -> p c", p=128))
            segall = consts.tile([128, NCH], f32, name="segall", tag="segall")
            nc.sync.dma_start(segall[:], seg_d[:].rearrange("(c p) -> p c", p=128))

            P_all = bigp.tile([128, NCH * REL], f32, name="P_all", tag="P_all")
            attst = bigp.tile([128, NCH * REL], f32, name="attst", tag="attst")
            xstage = bigp.tile([128, NCH], f32, name="xstage", tag="xstage")

            xT_v = xT_d[:].rearrange("(k p) t -> p k t", p=KCH)

            # ---------------- Stage A ----------------
            scA = nc.enter_named_scope("stageA", True)
            for j in range(NJ):
                xt = xtp.tile([KCH, NK, 512], dtmm, name="xt", tag="xt")
                nc.sync.dma_start(xt[:], xT_v[:, :, 512 * j:512 * (j + 1)])
                pt_ps = ptps.tile([REL, 512], f32, space="PSUM", name="pt_ps",
                                  tag="pt_ps")
                for k in range(NK):
                    nc.tensor.matmul(pt_ps[:], wm_sb[:, k, :], xt[:, k, :],
                                     start=(k == 0), stop=(k == NK - 1))
                pt_sb = ptsbp.tile([REL, 512], f32, name="pt_sb", tag="pt_sb")
                nc.vector.tensor_copy(pt_sb[:], pt_ps[:])
                for q in range(4):
                    c = 4 * j + q
                    tr = trps.tile([128, REL], f32, space="PSUM", name="tr", tag="tr")
                    nc.tensor.transpose(tr[:], pt_sb[:, 128 * q:128 * (q + 1)],
                                        ident[:REL, :REL])
                    nc.vector.tensor_copy(P_all[:, REL * c:REL * (c + 1)], tr[:])
                    mask = smallp.tile([128, REL], f32, name="mask", tag="mask")
                    nc.vector.tensor_scalar(mask[:], io53[:], laball[:, c:c + 1],
                                            None, Alu.is_equal)
                    junk = smallp.tile([128, REL], f32, name="junk", tag="junk")
                    nc.vector.tensor_tensor(
                        out=junk[:], in0=mask[:],
                        in1=P_all[:, REL * c:REL * (c + 1)], op=Alu.mult)
                    nc.vector.tensor_reduce(
                        xstage[:, c:c + 1], junk[:], mybir.AxisListType.X, Alu.add)
            nc.sync.dma_start(
                xlin_d[:].rearrange("p t -> (p t)").rearrange("(c q) -> q c", q=128),
                xstage[:])

            nc.leave_named_scope("stageA", scA[0], True)
            # ---------------- Stage B ----------------
            scB = nc.enter_named_scope("stageB", True)
            xf = segp.tile([128, C], f32, name="xf", tag="xf")
            nc.sync.dma_start(xf[:], xlin_d[:])
            cft = segp.tile([128, C], f32, name="cft", tag="cft")
            nc.sync.dma_start(cft[:], cf_d[:])
            cbt = segp.tile([128, C], f32, name="cbt", tag="cbt")
            nc.sync.dma_start(cbt[:], cb_d[:])
            xB = segp.tile([128, C], f32, name="xB", tag="xB")
            nc.vector.tensor_scalar_add(xB[:], xf[:], BSHIFT)

            def seg_scan(mask_t, data_t, op, rev, nm):
                dst = segp.tile([128, C], f32, name=nm, tag=nm)
                if rev:
                    o, mt, dd = dst[:, ::-1], mask_t[:, ::-1], data_t[:, ::-1]
                else:
                    o, mt, dd = dst[:], mask_t[:], data_t[:]
                nc.vector.tensor_tensor_scan(o, mt, dd, 0.0, Alu.mult, op)
                # cross-partition carry: partition p's slice may continue the
                # segment from partition p-1 (p+1 for rev). One step suffices
                # because a segment never covers a whole slice (len <= C).
                lcol = dst[:, 0:1] if rev else dst[:, C - 1:C]
                cry_d = dram.tile([128], f32, name=nm + "_cryd", tag=nm + "_cryd")
                nc.sync.dma_start(cry_d[:].rearrange("(p o) -> p o", o=1), lcol)
                lt_sb = segp.tile([1, 128], f32, name=nm + "_lts", tag=nm + "_lts")
                nc.sync.dma_start(lt_sb[:], cry_d[:].rearrange("(o p) -> o p", o=1))
                carr = segp.tile([1, 128], f32, name=nm + "_car", tag=nm + "_car")
                nc.vector.memset(carr[:], 0.0)
                if rev:
                    nc.vector.tensor_copy(carr[0:1, 0:127], lt_sb[0:1, 1:128])
                else:
                    nc.vector.tensor_copy(carr[0:1, 1:128], lt_sb[0:1, 0:127])
                cry2_d = dram.tile([128], f32, name=nm + "_cry2d", tag=nm + "_cry2d")
                nc.sync.dma_start(cry2_d[:].rearrange("(o p) -> o p", o=1), carr[:])
                ci = segp.tile([128, 1], f32, name=nm + "_ci", tag=nm + "_ci")
                nc.sync.dma_start(ci[:], cry2_d[:].rearrange("(p o) -> p o", o=1))
                nc.vector.tensor_tensor_scan(o, mt, dd, ci[:, 0:1], Alu.mult, op)
                return dst

            fmax = seg_scan(cft, xB, Alu.max, False, "fmax")
            bmax = seg_scan(cbt, xB, Alu.max, True, "bmax")
            mseg = segp.tile([128, C], f32, name="mseg", tag="mseg")
            nc.vector.tensor_tensor(out=mseg[:], in0=fmax[:], in1=bmax[:], op=Alu.max)
            dlt = segp.tile([128, C], f32, name="dlt", tag="dlt")
            nc.vector.tensor_tensor(out=dlt[:], in0=xB[:], in1=mseg[:], op=Alu.subtract)
            ev = segp.tile([128, C], f32, name="ev", tag="ev")
            nc.scalar.activation(ev[:], dlt[:], mybir.ActivationFunctionType.Exp)
            fs = seg_scan(cft, ev, Alu.add, False, "fs")
            bs = seg_scan(cbt, ev, Alu.add, True, "bs")
            den = segp.tile([128, C], f32, name="den", tag="den")
            nc.vector.tensor_tensor(out=den[:], in0=fs[:], in1=bs[:], op=Alu.add)
            den2 = segp.tile([128, C], f32, name="den2", tag="den2")
            nc.vector.tensor_tensor(out=den2[:], in0=den[:], in1=ev[:], op=Alu.subtract)
            rden = segp.tile([128, C], f32, name="rden", tag="rden")
            nc.vector.reciprocal(rden[:], den2[:])
            wv = segp.tile([128, C], f32, name="wv", tag="wv")
            nc.vector.tensor_tensor(out=wv[:], in0=ev[:], in1=rden[:], op=Alu.mult)
            nc.sync.dma_start(wlin_d[:], wv[:])

            nc.leave_named_scope("stageB", scB[0], True)
            # ---------------- Stage C ----------------
            scC = nc.enter_named_scope("stageC", True)
            wall = segp.tile([128, NCH], f32, name="wall", tag="wall")
            nc.sync.dma_start(
                wall[:],
                wlin_d[:].rearrange("p t -> (p t)").rearrange("(c q) -> q c", q=128))
            for c in range(NCH):
                Ht = hbp.tile([128, 128], f32, name="Ht", tag="Ht")
                nc.vector.tensor_scalar(Ht[:], io128[:], segall[:, c:c + 1],
                                        wall[:, c:c + 1], Alu.is_equal, Alu.mult)
                ops = trps.tile([128, REL], f32, space="PSUM", name="ops", tag="tr")
                nc.tensor.matmul(ops[:], Ht[:], P_all[:, REL * c:REL * (c + 1)],
                                 start=True, stop=True)
                nc.vector.tensor_copy(attst[:, REL * c:REL * (c + 1)], ops[:])
            nc.sync.dma_start(
                att_d[:].rearrange("c p r -> p c r"),
                attst[:].rearrange("p (c r) -> p c r", r=REL))
            nc.leave_named_scope("stageC", scC[0], True)

    nc.compile()
    return nc


def _prep(repre, relation_mat, bias, scope, labels, ncores):
    repre = np.ascontiguousarray(np.asarray(repre, dtype=np.float32))
    relmat = np.asarray(relation_mat, dtype=np.float32)
    bias_np = np.asarray(bias, dtype=np.float32)
    scope = np.asarray(scope).astype(np.int64)
    labels_np = np.asarray(labels).astype(np.int64)
    n, d = repre.shape
    nbags = scope.shape[0]
    assert d == DIM and nbags % ncores == 0
    bpc = nbags // ncores
    starts, ends = scope[:, 0], scope[:, 1]
    lens = ends - starts
    core_r0 = starts[np.arange(ncores) * bpc]
    core_r1 = ends[np.arange(ncores) * bpc + bpc - 1]
    rows = core_r1 - core_r0
    Rpad = int(512 * math.ceil(int(rows.max()) / 512))
    C = Rpad // 128
    assert int(lens.max()) <= min(128, C), "bag too large for this kernel layout"

    wm = np.empty((KCH, NK, REL), np.float32)
    for k in range(NK):
        wm[:, k, :] = relmat[:, k * KCH:(k + 1) * KCH].T

    in_maps, metas = [], []
    for c in range(ncores):
        r0, r1 = int(core_r0[c]), int(core_r1[c])
        rc = r1 - r0
        xT = np.zeros((d, Rpad), np.float32)
        xT[:, :rc] = repre[r0:r1].T
        labf = np.zeros(Rpad, np.float32)
        labf[:rc] = labels_np[r0:r1]
        blens = lens[c * bpc:(c + 1) * bpc]
        segl = np.repeat(np.arange(bpc, dtype=np.int64), blens)
        seg_pad = np.concatenate(
            [segl, bpc + np.arange(Rpad - rc, dtype=np.int64)])
        cf_lin = np.ones(Rpad, np.float32)
        cf_lin[0] = 0.0
        cf_lin[1:] = (seg_pad[1:] == seg_pad[:-1]).astype(np.float32)
        cb_lin = np.zeros(Rpad, np.float32)
        cb_lin[:-1] = (seg_pad[:-1] == seg_pad[1:]).astype(np.float32)
        chunk_first = seg_pad[(np.arange(Rpad) // 128) * 128]
        seg_local = (seg_pad - chunk_first).astype(np.float32)
        assert seg_local.max() <= 127
        in_maps.append({
            "xT": xT, "wm": wm, "labf": labf, "segloc": seg_local,
            "cf": np.ascontiguousarray(cf_lin.reshape(128, C)),
            "cb": np.ascontiguousarray(cb_lin.reshape(128, C)),
        })
        ls = starts[c * bpc:(c + 1) * bpc] - r0
        le = ends[c * bpc:(c + 1) * bpc] - r0
        k0 = ls // 128
        k1 = (le - 1) // 128
        bidx = np.arange(bpc, dtype=np.int64)
        slot0 = bidx - chunk_first[k0 * 128]
        slot1 = bidx - chunk_first[k1 * 128]
        assert slot0.min() >= 0 and slot0.max() <= 127
        assert slot1.min() >= 0 and slot1.max() <= 127
        metas.append((k0, slot0, k1, slot1))
    return in_maps, metas, bias_np, Rpad, bpc


def _compact(results, metas, bias_np, bpc):
    out = np.empty((len(results) * bpc, REL), np.float32)
    for c, res in enumerate(results):
        stage = res["attstage"]
        k0, slot0, k1, slot1 = metas[c]
        att = stage[k0, slot0, :].astype(np.float32, copy=True)
        two = k1 > k0
        att[two] += stage[k1[two], slot1[two], :]
        out[c * bpc:(c + 1) * bpc] = att
    out += bias_np[None, :]
    return out


def kernel(repre, relation_mat, bias, scope, labels):
    global LAST_RESULTS
    from concourse.bass_utils import run_bass_kernel_spmd

    in_maps, metas, bias_np, Rpad, bpc = _prep(
        repre, relation_mat, bias, scope, labels, NCORES)
    if Rpad not in _PROGRAM_CACHE:
        _PROGRAM_CACHE[Rpad] = _build_program(Rpad)
    nc = _PROGRAM_CACHE[Rpad]
    res = run_bass_kernel_spmd(nc, in_maps, core_ids=list(range(NCORES)),
                               trace=bool(os.environ.get("BASS_TRACE")))
    LAST_RESULTS = res
    return _compact(res.results, metas, bias_np, bpc)

